# revision 12
# baseline (speedup 1.0000x reference)
"""Trainium2 Bass kernel for a 2-stage 13-organ Dice loss (fp8 edition).

Math (all organ weights are 1.0, so the per-organ fold collapses to sums):
  for stage s, batch b:
    num[s,b] = 2 * sum_{c in 1..13} sum_v pred_s[b,c,v] * [target[b,v]==c]
    den[s,b] = sum_{c in 1..13} sum_v pred_s[b,c,v]^2 + count(target[b]!=0) + 13*EPS
  dice[b] = num[1,b]/den[1,b] + num[2,b]/den[2,b]
  loss    = mean_b(2 - dice[b])

Sharding: the 48-slice depth axis is split 6-per-core across 8 NeuronCores;
each core handles both batches, both stages, and organ channels 1..13
(channel 0 is background and never touches the device).

The kernel streams pred in fp8 e4m3 (host-side cast; values are uniform in
[0,1) so TRN fp8e4 == OCP e4m3fn here). The loss is a ratio of sums over
~40M elements, so fp8 quantization noise averages down to ~2e-4 relative
on the final scalar (vs a 2e-2 gate). Halving the bytes halves the DMA
time, which was the baseline bottleneck; the engines are now the limit, so
the per-element work is spread across all three compute engines:

  - DVE builds the 13 one-hot masks per depth block in bf16 (4x perf
    mode) plus a zero-count accumulation.
  - PE computes the full numerator: each bf16 mask chunk is the stationary
    (FWL load), multiplied against BOTH stages' fp8 pred chunks as one
    N=256 moving operand; per-(s,b) PSUM diagonals hold sum(pred*onehot).
  - The denominator sum-of-squares is split three ways by column range:
      ACT: activation(Square) with fused f32 accum
      DVE: scalar_tensor_tensor p*p with fused accum
      PE:  fp8 DoubleRow self-matmul — adjacent 128-col chunk pairs as
           [K,2,128] APs; the PSUM diagonal accumulates the sum of squares
           of both chunks at 2 elems/cycle
    (1 unit = 256 pred columns; per-block unit split in SPLITS below.)

The depth dimension is processed as variable-size blocks: batch 0 starts
with two single-slice blocks so the first pred DMA is half size and the
engines start ~5us earlier; steady state uses 2-slice blocks.

All reductions land in f32 slot tiles / PSUM blocks that are DMA'd out;
the host does the tiny final reduction and the dice division.
"""

import numpy as np
import ml_dtypes

import concourse.bacc as bacc
import concourse.mybir as mybir
import concourse.tile as tile
from concourse.bass_utils import run_bass_kernel_spmd

N_CORES = 8
S = 2  # stages
B = 2  # batch
C = 13  # organ channels (pred channels 1..13; channel 0 skipped)
D = 48  # depth
D_SH = D // N_CORES  # 6 depth slices per core
HW = 256 * 256  # voxels per (b, d) slab
PJ = HW // 128  # 512 free elems per partition per slab
U = 256  # columns per work unit (= one DoubleRow chunk pair)
# Depth blocks per core: (batch, depth offset, n slices). First two blocks
# are single-slice so the pipeline fills quickly.
BLOCKS = [(0, 0, 1), (0, 1, 1), (0, 2, 2), (0, 4, 2),
          (1, 0, 2), (1, 2, 2), (1, 4, 1), (1, 5, 1)]
# Per-dg denominator split (ACT units, DVE units, PE units); units of 256
# columns out of 26*dg total per (s, block). Chosen so ACT / DVE / PE land
# near the same busy time: ACT ~1.2 elem/ns, DVE fp8 STT ~0.95 elem/ns on
# top of ~40us of mask/count work, PE ~2.4 cols/ns DoubleRow on top of
# ~70us of numerator matmuls.
SPLITS = {1: (14, 6, 6), 2: (28, 13, 11)}
EPS = 1e-5

F32 = mybir.dt.float32
BF16 = mybir.dt.bfloat16
FP8 = mybir.dt.float8e4

TOTF = sum(S * C * dg * PJ for _, _, dg in BLOCKS)  # flat pred bytes/partition


def build_program(blocks=BLOCKS) -> bacc.Bacc:
    """Build the per-core SPMD Bass program (fp8 pred, bf16 target).

    The host pre-packs inputs into the exact SBUF layout so every DMA is a
    fully contiguous block:
      pred [128, TOTF] fp8 — per block (b,d0,dg) a [S, C, dg*PJ] slab,
        where the dg*PJ axis is element [d*PJ + j] = voxel (p*PJ+j) of
        depth slice d0+d
      tgt  [B, 128, D_SH*PJ] bf16 — element [b, p, d*PJ + j]

    Outputs (per core):
      onum [128, 256*B] f32 — per-b numerator PSUM blocks; cols
        [b*256 + s*128, +128) hold M[i,j] = sum over chunks of
        sum_p mask[p,i]*pred_s[p,j]; the DIAGONAL sums to sum(pred*onehot).
      osq  [128, 256*B] f32 — per-(b,s) DoubleRow self-matmul PSUM blocks;
        the diagonal sums to sum(pred^2) of the PE-assigned units.
      oden [128, 32] f32 (slot blk*S+s: ACT per-partition square sums)
      osl  [128, 64] f32 (col 32+blk*S+s: DVE per-partition square sums)
      ocnt [128, 16] f32 (slot blk: per-partition counts of target==0)
    """
    nblk = len(blocks)
    assert nblk * S <= 32
    nc = bacc.Bacc(target_bir_lowering=False)
    pred = nc.dram_tensor("pred", [128, TOTF], FP8, kind="ExternalInput")
    tgt = nc.dram_tensor("tgt", [B, 128, D_SH * PJ], BF16, kind="ExternalInput")
    onum = nc.dram_tensor("onum", [128, 256 * B], F32, kind="ExternalOutput")
    osq = nc.dram_tensor("osq", [128, 256 * B], F32, kind="ExternalOutput")
    oden = nc.dram_tensor("oden", [128, 32], F32, kind="ExternalOutput")
    osl = nc.dram_tensor("osl", [128, 64], F32, kind="ExternalOutput")
    ocnt = nc.dram_tensor("ocnt", [128, 16], F32, kind="ExternalOutput")
    num_total = {
        b: sum(C * dg * PJ // 128 for bb, _, dg in blocks if bb == b)
        for b in range(B)
    }
    sq_total = {
        b: sum(SPLITS[dg][2] for bb, _, dg in blocks if bb == b) for b in range(B)
    }

    with tile.TileContext(nc) as tc:
        with (
            tc.tile_pool(name="tpool", bufs=2) as tpool,
            tc.tile_pool(name="ppool", bufs=3) as ppool,
            tc.tile_pool(name="mpool", bufs=3) as mpool,
            tc.tile_pool(name="dpool", bufs=1) as dpool,
            tc.tile_pool(name="spool", bufs=1) as spool,
            tc.tile_pool(name="qpool", bufs=1, space="PSUM") as qpool,
        ):
            den_slots = spool.tile([128, 32], F32, tag="den")
            sl_slots = spool.tile([128, 64], F32, tag="sl")
            cnt_slots = spool.tile([128, 16], F32, tag="cnt")
            nc.vector.memset(den_slots[:, :], 0.0)
            nc.vector.memset(sl_slots[:, :], 0.0)
            nc.vector.memset(cnt_slots[:, :], 0.0)
            # Each PSUM accumulation group gets its own full 2KB bank: the
            # hardware's start_tensor_calc pending-zero granularity spans
            # the bank, so two concurrently-open groups in one bank clobber
            # each other's partials.
            pnum = {
                b: qpool.tile([128, 512], F32, tag=f"pn{b}", name=f"pnum_{b}")
                for b in range(B)
            }
            psq = {
                (b, s): qpool.tile([128, 512], F32, tag=f"pq{b}{s}", name=f"psq_{b}{s}")
                for b in range(B)
                for s in range(S)
            }
            nmm = {b: 0 for b in range(B)}
            sqm = {(b, s): 0 for b in range(B) for s in range(S)}

            cur_tb = {}
            off = 0
            for blk, (b, d0, dg) in enumerate(blocks):
                if b not in cur_tb:
                    cur_tb[b] = tpool.tile(
                        [128, D_SH * PJ], BF16, tag="tb", name=f"tb_{b}"
                    )
                    nc.sync.dma_start(out=cur_tb[b][:, :], in_=tgt[b])
                tb = cur_tb[b]
                ub = 2 * C * dg  # total units this block (per stage)
                ca, cd, cp = SPLITS[dg]
                blkf = S * C * dg * PJ
                tsl = tb[:, d0 * PJ : (d0 + dg) * PJ]
                # One DMA brings BOTH stages' pred slab for this block.
                pt = ppool.tile([128, S, 2 * C * 2, 2, 128], FP8, tag="pt")
                nc.sync.dma_start(
                    out=pt[:, :, :ub, :, :], in_=pred[:, off : off + blkf]
                )
                off += blkf
                # 13 one-hot masks (bf16 in/out -> 4x DVE mode), shaped to
                # match pred chunks.
                masks = mpool.tile([128, C, 8, 128], BF16, tag="masks")
                for c in range(C):
                    nc.vector.tensor_scalar(
                        masks[:, c, : dg * 4, :],
                        tsl,
                        float(c + 1),
                        None,
                        mybir.AluOpType.is_equal,
                    )
                zdummy = dpool.tile([128, 2 * PJ], BF16, tag="zd")
                nc.vector.tensor_scalar(
                    zdummy[:, : dg * PJ],
                    tsl,
                    0.0,
                    None,
                    mybir.AluOpType.is_equal,
                    mybir.AluOpType.add,
                    accum_out=cnt_slots[:, blk : blk + 1],
                )
                for s in range(S):
                    slot = blk * S + s
                    # Denominator squares, ACT share: one big fused op.
                    sdummy = dpool.tile([128, SPLITS[2][0] * U], FP8, tag="sd")
                    nc.scalar.activation(
                        sdummy[:, : ca * U],
                        pt[:, s, :ca, :, :],
                        mybir.ActivationFunctionType.Square,
                        accum_out=den_slots[:, slot : slot + 1],
                    )
                    # DVE share via scalar_tensor_tensor p*1*p.
                    sdummy2 = dpool.tile([128, SPLITS[2][1] * U], FP8, tag="sd2")
                    nc.vector.scalar_tensor_tensor(
                        out=sdummy2[:, : cd * U],
                        in0=pt[:, s, ca : ca + cd, :, :],
                        scalar=1.0,
                        in1=pt[:, s, ca : ca + cd, :, :],
                        op0=mybir.AluOpType.mult,
                        op1=mybir.AluOpType.mult,
                        accum_out=sl_slots[:, 32 + slot : 32 + slot + 1],
                    )
                # Numerator on TensorE: load each mask chunk as the
                # stationary ONCE and stream both stages' fp8 pred chunks
                # as one N=256 moving operand; accumulate into the per-b
                # PSUM block (host extracts the diagonals).
                for c in range(C):
                    for k in range(dg * 4):
                        nmm[b] += 1
                        nc.tensor.matmul(
                            pnum[b][:128, : S * 128],
                            masks[:, c, k, :],
                            pt[:, :, c * dg * 2 + k // 2, k % 2, :],
                            start=(nmm[b] == 1),
                            stop=(nmm[b] == num_total[b]),
                        )
                # PE share of the squares: fp8 DoubleRow self-matmul over
                # chunk pairs; the (b,s) PSUM diagonal accumulates
                # sum(p^2) of both chunks of each unit.
                for s in range(S):
                    for u in range(ca + cd, ub):
                        sqm[(b, s)] += 1
                        nc.tensor.matmul(
                            psq[(b, s)][:128, :128],
                            pt[:, s, u, :, :],
                            pt[:, s, u, :, :],
                            start=(sqm[(b, s)] == 1),
                            stop=(sqm[(b, s)] == sq_total[b]),
                            perf_mode=mybir.MatmulPerfMode.DoubleRow,
                        )

            numsb = spool.tile([128, 256 * B], F32, tag="numsb")
            sqsb = spool.tile([128, 256 * B], F32, tag="sqsb")
            for b in range(B):
                nc.vector.tensor_copy(
                    numsb[:, b * 256 : (b + 1) * 256], pnum[b][:, :256]
                )
                for s in range(S):
                    nc.vector.tensor_copy(
                        sqsb[:, b * 256 + s * 128 : b * 256 + (s + 1) * 128],
                        psq[(b, s)][:, :128],
                    )
            nc.sync.dma_start(out=onum[:, :], in_=numsb[:, :])
            nc.sync.dma_start(out=osq[:, :], in_=sqsb[:, :])
            nc.sync.dma_start(out=oden[:, :], in_=den_slots[:, :])
            nc.sync.dma_start(out=osl[:, :], in_=sl_slots[:, :])
            nc.sync.dma_start(out=ocnt[:, :], in_=cnt_slots[:, :])
    nc.finalize()
    return nc


def shard_inputs(pred_stage1, pred_stage2, target, n_cores=N_CORES):
    """Slice off the background channel, split depth per core, cast pred to
    fp8 e4m3 / target to bf16, and pack into the device layout."""
    in_maps = []
    p1 = np.asarray(pred_stage1)
    p2 = np.asarray(pred_stage2)
    tg = np.asarray(target)
    for k in range(n_cores):
        base = k * D_SH
        flat = np.empty((128, TOTF), ml_dtypes.float8_e4m3fn)
        off = 0
        for b, d0, dg in BLOCKS:
            blkf = C * dg * PJ
            for s, src in enumerate((p1, p2)):
                x = src[b, 1:, base + d0 : base + d0 + dg]  # (C, dg, 256, 256)
                x = x.reshape(C, dg, 128, PJ).transpose(2, 0, 1, 3)
                flat[:, off : off + blkf] = x.reshape(128, blkf)
                off += blkf
        t = tg[:, base : base + D_SH].reshape(B, D_SH, 128, PJ).transpose(0, 2, 1, 3)
        tshard = t.reshape(B, 128, D_SH * PJ).astype(ml_dtypes.bfloat16)
        in_maps.append({"pred": flat, "tgt": tshard})
    return in_maps


def combine_results(results):
    """Host-side final reduction of the per-core per-partition partials."""
    num = np.zeros((S, B), np.float64)
    den = np.zeros((S, B), np.float64)
    cnt = np.zeros((B,), np.float64)
    for r in results:
        onum = r["onum"].astype(np.float64)
        osq = r["osq"].astype(np.float64)
        oden = r["oden"].astype(np.float64)
        osl = r["osl"].astype(np.float64)
        ocnt = r["ocnt"].astype(np.float64)
        for b in range(B):
            for s in range(S):
                blkc = slice(b * 256 + s * 128, b * 256 + (s + 1) * 128)
                num[s, b] += 2.0 * np.trace(onum[:, blkc])
                den[s, b] += np.trace(osq[:, blkc])
        for blk, (b, d0, dg) in enumerate(BLOCKS):
            cnt[b] += 128 * PJ * dg - ocnt[:, blk].sum()
            for s in range(S):
                slot = blk * S + s
                den[s, b] += oden[:, slot].sum() + osl[:, 32 + slot].sum()
    dice = np.zeros(B, np.float64)
    for b in range(B):
        for s in range(S):
            dice[b] += num[s, b] / (den[s, b] + cnt[b] + C * EPS)
    loss = np.mean(2.0 - dice)
    return np.array(loss, dtype=np.float32)


def kernel(pred_stage1, pred_stage2, target):
    in_maps = shard_inputs(pred_stage1, pred_stage2, target)
    nc = build_program()
    # The first multi-core execution of a freshly loaded NEFF occasionally
    # hits a transient NRT_EXEC_UNIT_UNRECOVERABLE; a retry succeeds.
    last_err = None
    for _ in range(3):
        try:
            res = run_bass_kernel_spmd(nc, in_maps, list(range(N_CORES)))
            return combine_results(res.results)
        except Exception as e:  # noqa: BLE001
            last_err = e
    raise last_err


# revision 17
# speedup vs baseline: 1.0196x; 1.0196x over previous
"""Trainium2 Bass kernel for a 2-stage 13-organ Dice loss (fp8 edition).

Math (all organ weights are 1.0, so the per-organ fold collapses to sums):
  for stage s, batch b:
    num[s,b] = 2 * sum_{c in 1..13} sum_v pred_s[b,c,v] * [target[b,v]==c]
    den[s,b] = sum_{c in 1..13} sum_v pred_s[b,c,v]^2 + count(target[b]!=0) + 13*EPS
  dice[b] = num[1,b]/den[1,b] + num[2,b]/den[2,b]
  loss    = mean_b(2 - dice[b])

Sharding: the 48-slice depth axis is split 6-per-core across 8 NeuronCores;
each core handles both batches, both stages, and organ channels 1..13
(channel 0 is background and never touches the device).

The kernel streams pred in fp8 e4m3 (host-side cast; values are uniform in
[0,1) so TRN fp8e4 == OCP e4m3fn here). The loss is a ratio of sums over
~40M elements, so fp8 quantization noise averages down to ~2e-4 relative
on the final scalar (vs a 2e-2 gate). Halving the bytes halves the DMA
time, which was the baseline bottleneck; the engines are now the limit, so
the per-element work is spread across all three compute engines:

  - DVE builds the 13 one-hot masks per depth block in bf16 (4x perf
    mode) plus a zero-count accumulation.
  - PE computes the full numerator: each bf16 mask chunk is the stationary
    (FWL load), multiplied against BOTH stages' fp8 pred chunks as one
    N=256 moving operand; per-(s,b) PSUM diagonals hold sum(pred*onehot).
  - The denominator sum-of-squares is split three ways by column range:
      ACT: activation(Square) with fused f32 accum
      DVE: scalar_tensor_tensor p*p with fused accum
      PE:  fp8 DoubleRow self-matmul — adjacent 128-col chunk pairs as
           [K,2,128] APs; the PSUM diagonal accumulates the sum of squares
           of both chunks at 2 elems/cycle
    (1 unit = 256 pred columns; per-block unit split in SPLITS below.)

The depth dimension is processed as variable-size blocks: batch 0 starts
with two single-slice blocks so the first pred DMA is half size and the
engines start ~5us earlier; steady state uses 2-slice blocks.

All reductions land in f32 slot tiles / PSUM blocks that are DMA'd out;
the host does the tiny final reduction and the dice division.
"""

import numpy as np
import ml_dtypes

import concourse.bacc as bacc
import concourse.mybir as mybir
import concourse.tile as tile
from concourse.bass_utils import run_bass_kernel_spmd

N_CORES = 8
S = 2  # stages
B = 2  # batch
C = 13  # organ channels (pred channels 1..13; channel 0 skipped)
D = 48  # depth
D_SH = D // N_CORES  # 6 depth slices per core
HW = 256 * 256  # voxels per (b, d) slab
PJ = HW // 128  # 512 free elems per partition per slab
U = 256  # columns per work unit (= one DoubleRow chunk pair)
# Depth blocks per core: (batch, depth offset, n slices). First two blocks
# are single-slice so the pipeline fills quickly.
BLOCKS = [(0, 0, 1), (0, 1, 1), (0, 2, 2), (0, 4, 2),
          (1, 0, 2), (1, 2, 2), (1, 4, 2)]
# Per-dg denominator split (ACT units, DVE units, PE units); units of 256
# columns out of 26*dg total per (s, block). Chosen so ACT / DVE / PE land
# near the same busy time: ACT ~1.2 elem/ns, DVE fp8 STT ~0.95 elem/ns on
# top of ~40us of mask/count work, PE ~2.4 cols/ns DoubleRow on top of
# ~70us of numerator matmuls.
SPLITS = {1: (14, 6, 6), 2: (28, 13, 11)}
EPS = 1e-5

F32 = mybir.dt.float32
BF16 = mybir.dt.bfloat16
FP8 = mybir.dt.float8e4

TOTF = sum(S * C * dg * PJ for _, _, dg in BLOCKS)  # flat pred bytes/partition


def build_program(blocks=BLOCKS) -> bacc.Bacc:
    """Build the per-core SPMD Bass program (fp8 pred, bf16 target).

    The host pre-packs inputs into the exact SBUF layout so every DMA is a
    fully contiguous block:
      pred [128, TOTF] fp8 — per block (b,d0,dg) a [S, C, dg*PJ] slab,
        where the dg*PJ axis is element [d*PJ + j] = voxel (p*PJ+j) of
        depth slice d0+d
      tgt  [B, 128, D_SH*PJ] bf16 — element [b, p, d*PJ + j]

    Outputs (per core):
      onum [128, 256*B] f32 — per-b numerator PSUM blocks; cols
        [b*256 + s*128, +128) hold M[i,j] = sum over chunks of
        sum_p mask[p,i]*pred_s[p,j]; the DIAGONAL sums to sum(pred*onehot).
      osq  [128, 256*B] f32 — per-(b,s) DoubleRow self-matmul PSUM blocks;
        the diagonal sums to sum(pred^2) of the PE-assigned units.
      oden [128, 32] f32 (slot blk*S+s: ACT per-partition square sums)
      osl  [128, 64] f32 (col 32+blk*S+s: DVE per-partition square sums)
      ocnt [128, 16] f32 (slot blk: per-partition counts of target==0)
    """
    nblk = len(blocks)
    assert nblk * S <= 32
    nc = bacc.Bacc(target_bir_lowering=False)
    pred = nc.dram_tensor("pred", [128, TOTF], FP8, kind="ExternalInput")
    tgt = nc.dram_tensor("tgt", [B, 128, D_SH * PJ], BF16, kind="ExternalInput")
    onum = nc.dram_tensor("onum", [128, 256 * B], F32, kind="ExternalOutput")
    osq = nc.dram_tensor("osq", [128, 256 * B], F32, kind="ExternalOutput")
    oden = nc.dram_tensor("oden", [128, 32], F32, kind="ExternalOutput")
    osl = nc.dram_tensor("osl", [128, 64], F32, kind="ExternalOutput")
    ocnt = nc.dram_tensor("ocnt", [128, 16], F32, kind="ExternalOutput")
    num_total = {
        b: sum(C * dg * PJ // 128 for bb, _, dg in blocks if bb == b)
        for b in range(B)
    }
    sq_total = {
        b: sum(SPLITS[dg][2] for bb, _, dg in blocks if bb == b) for b in range(B)
    }

    with tile.TileContext(nc) as tc:
        with (
            tc.tile_pool(name="tpool", bufs=2) as tpool,
            tc.tile_pool(name="ppool", bufs=4) as ppool,
            tc.tile_pool(name="mpool", bufs=2) as mpool,
            tc.tile_pool(name="dpool", bufs=1) as dpool,
            tc.tile_pool(name="spool", bufs=1) as spool,
            tc.tile_pool(name="qpool", bufs=1, space="PSUM") as qpool,
        ):
            den_slots = spool.tile([128, 32], F32, tag="den")
            sl_slots = spool.tile([128, 64], F32, tag="sl")
            cnt_slots = spool.tile([128, 16], F32, tag="cnt")
            numsb = spool.tile([128, 256 * B], F32, tag="numsb")
            sqsb = spool.tile([128, 256 * B], F32, tag="sqsb")
            nc.vector.memset(den_slots[:, :], 0.0)
            nc.vector.memset(sl_slots[:, :], 0.0)
            nc.vector.memset(cnt_slots[:, :], 0.0)
            # Each PSUM accumulation group gets its own full 2KB bank: the
            # hardware's start_tensor_calc pending-zero granularity spans
            # the bank, so two concurrently-open groups in one bank clobber
            # each other's partials.
            pnum = {
                b: qpool.tile([128, 512], F32, tag=f"pn{b}", name=f"pnum_{b}")
                for b in range(B)
            }
            psq = {
                (b, s): qpool.tile([128, 512], F32, tag=f"pq{b}{s}", name=f"psq_{b}{s}")
                for b in range(B)
                for s in range(S)
            }
            nmm = {b: 0 for b in range(B)}
            sqm = {(b, s): 0 for b in range(B) for s in range(S)}

            cur_tb = {}
            for b in range(B):
                cur_tb[b] = tpool.tile(
                    [128, D_SH * PJ], BF16, tag="tb", name=f"tb_{b}"
                )
                nc.sync.dma_start(out=cur_tb[b][:, :], in_=tgt[b])

            def emit_masks(blk):
                # 13 one-hot masks (bf16 in/out -> 4x DVE mode), shaped to
                # match pred chunks, plus the zero-count accumulation.
                b, d0, dg = blocks[blk]
                tsl = cur_tb[b][:, d0 * PJ : (d0 + dg) * PJ]
                masks = mpool.tile(
                    [128, C, 8, 128], BF16, tag="masks", name=f"masks_{blk}"
                )
                for c in range(C):
                    nc.vector.tensor_scalar(
                        masks[:, c, : dg * 4, :],
                        tsl,
                        float(c + 1),
                        None,
                        mybir.AluOpType.is_equal,
                    )
                zdummy = dpool.tile([128, 2 * PJ], BF16, tag="zd")
                nc.vector.tensor_scalar(
                    zdummy[:, : dg * PJ],
                    tsl,
                    0.0,
                    None,
                    mybir.AluOpType.is_equal,
                    mybir.AluOpType.add,
                    accum_out=cnt_slots[:, blk : blk + 1],
                )
                return masks

            pending_masks = emit_masks(0)
            off = 0
            for blk, (b, d0, dg) in enumerate(blocks):
                ub = 2 * C * dg  # total units this block (per stage)
                ca, cd, cp = SPLITS[dg]
                blkf = S * C * dg * PJ
                # One DMA brings BOTH stages' pred slab for this block.
                pt = ppool.tile([128, S, 2 * C * 2, 2, 128], FP8, tag="pt")
                nc.sync.dma_start(
                    out=pt[:, :, :ub, :, :], in_=pred[:, off : off + blkf]
                )
                off += blkf
                masks = pending_masks
                # Build the NEXT block's masks before this block's DVE STT
                # share so the tensor engine never starves on masks.
                if blk + 1 < len(blocks):
                    pending_masks = emit_masks(blk + 1)
                for s in range(S):
                    slot = blk * S + s
                    # Denominator squares, ACT share: one big fused op.
                    sdummy = dpool.tile([128, SPLITS[2][0] * U], FP8, tag="sd")
                    nc.scalar.activation(
                        sdummy[:, : ca * U],
                        pt[:, s, :ca, :, :],
                        mybir.ActivationFunctionType.Square,
                        accum_out=den_slots[:, slot : slot + 1],
                    )
                    # DVE share via scalar_tensor_tensor p*1*p.
                    sdummy2 = dpool.tile([128, SPLITS[2][1] * U], FP8, tag="sd2")
                    nc.vector.scalar_tensor_tensor(
                        out=sdummy2[:, : cd * U],
                        in0=pt[:, s, ca : ca + cd, :, :],
                        scalar=1.0,
                        in1=pt[:, s, ca : ca + cd, :, :],
                        op0=mybir.AluOpType.mult,
                        op1=mybir.AluOpType.mult,
                        accum_out=sl_slots[:, 32 + slot : 32 + slot + 1],
                    )
                # Numerator on TensorE: load each mask chunk as the
                # stationary ONCE and stream both stages' fp8 pred chunks
                # as one N=256 moving operand; accumulate into the per-b
                # PSUM block (host extracts the diagonals).
                for c in range(C):
                    for k in range(dg * 4):
                        nmm[b] += 1
                        nc.tensor.matmul(
                            pnum[b][:128, : S * 128],
                            masks[:, c, k, :],
                            pt[:, :, c * dg * 2 + k // 2, k % 2, :],
                            start=(nmm[b] == 1),
                            stop=(nmm[b] == num_total[b]),
                        )
                # PE share of the squares: fp8 DoubleRow self-matmul over
                # chunk pairs; the (b,s) PSUM diagonal accumulates
                # sum(p^2) of both chunks of each unit.
                for s in range(S):
                    for u in range(ca + cd, ub):
                        sqm[(b, s)] += 1
                        nc.tensor.matmul(
                            psq[(b, s)][:128, :128],
                            pt[:, s, u, :, :],
                            pt[:, s, u, :, :],
                            start=(sqm[(b, s)] == 1),
                            stop=(sqm[(b, s)] == sq_total[b]),
                            perf_mode=mybir.MatmulPerfMode.DoubleRow,
                        )
                # Drain this b's PSUM blocks as soon as its last block is
                # done, overlapping the copies with the next b's compute.
                if nmm[b] == num_total[b]:
                    nc.vector.tensor_copy(
                        numsb[:, b * 256 : (b + 1) * 256], pnum[b][:, :256]
                    )
                    for s in range(S):
                        nc.vector.tensor_copy(
                            sqsb[:, b * 256 + s * 128 : b * 256 + (s + 1) * 128],
                            psq[(b, s)][:, :128],
                        )

            nc.sync.dma_start(out=onum[:, :], in_=numsb[:, :])
            nc.sync.dma_start(out=osq[:, :], in_=sqsb[:, :])
            nc.sync.dma_start(out=oden[:, :], in_=den_slots[:, :])
            nc.sync.dma_start(out=osl[:, :], in_=sl_slots[:, :])
            nc.sync.dma_start(out=ocnt[:, :], in_=cnt_slots[:, :])
    nc.finalize()
    return nc


def shard_inputs(pred_stage1, pred_stage2, target, n_cores=N_CORES):
    """Slice off the background channel, split depth per core, cast pred to
    fp8 e4m3 / target to bf16, and pack into the device layout."""
    in_maps = []
    p1 = np.asarray(pred_stage1)
    p2 = np.asarray(pred_stage2)
    tg = np.asarray(target)
    for k in range(n_cores):
        base = k * D_SH
        flat = np.empty((128, TOTF), ml_dtypes.float8_e4m3fn)
        off = 0
        for b, d0, dg in BLOCKS:
            blkf = C * dg * PJ
            for s, src in enumerate((p1, p2)):
                x = src[b, 1:, base + d0 : base + d0 + dg]  # (C, dg, 256, 256)
                x = x.reshape(C, dg, 128, PJ).transpose(2, 0, 1, 3)
                flat[:, off : off + blkf] = x.reshape(128, blkf)
                off += blkf
        t = tg[:, base : base + D_SH].reshape(B, D_SH, 128, PJ).transpose(0, 2, 1, 3)
        tshard = t.reshape(B, 128, D_SH * PJ).astype(ml_dtypes.bfloat16)
        in_maps.append({"pred": flat, "tgt": tshard})
    return in_maps


def combine_results(results):
    """Host-side final reduction of the per-core per-partition partials."""
    num = np.zeros((S, B), np.float64)
    den = np.zeros((S, B), np.float64)
    cnt = np.zeros((B,), np.float64)
    for r in results:
        onum = r["onum"].astype(np.float64)
        osq = r["osq"].astype(np.float64)
        oden = r["oden"].astype(np.float64)
        osl = r["osl"].astype(np.float64)
        ocnt = r["ocnt"].astype(np.float64)
        for b in range(B):
            for s in range(S):
                blkc = slice(b * 256 + s * 128, b * 256 + (s + 1) * 128)
                num[s, b] += 2.0 * np.trace(onum[:, blkc])
                den[s, b] += np.trace(osq[:, blkc])
        for blk, (b, d0, dg) in enumerate(BLOCKS):
            cnt[b] += 128 * PJ * dg - ocnt[:, blk].sum()
            for s in range(S):
                slot = blk * S + s
                den[s, b] += oden[:, slot].sum() + osl[:, 32 + slot].sum()
    dice = np.zeros(B, np.float64)
    for b in range(B):
        for s in range(S):
            dice[b] += num[s, b] / (den[s, b] + cnt[b] + C * EPS)
    loss = np.mean(2.0 - dice)
    return np.array(loss, dtype=np.float32)


def kernel(pred_stage1, pred_stage2, target):
    in_maps = shard_inputs(pred_stage1, pred_stage2, target)
    nc = build_program()
    # The first multi-core execution of a freshly loaded NEFF occasionally
    # hits a transient NRT_EXEC_UNIT_UNRECOVERABLE; a retry succeeds.
    last_err = None
    for _ in range(3):
        try:
            res = run_bass_kernel_spmd(nc, in_maps, list(range(N_CORES)))
            return combine_results(res.results)
        except Exception as e:  # noqa: BLE001
            last_err = e
    raise last_err
